# revision 32
# baseline (speedup 1.0000x reference)
"""Trainium2 Bass kernel for nn_LinearRNN (B=16, T=4096, D_in=256, H=512, D_out=256).

  xp = x @ W_in.T                       [B, T, H]
  h_t = xp_t + h_{t-1} @ W_h.T          (W_h is diagonal -> elementwise scan)
  out = hs @ W_out.T                    [B, T, D_out]

Strategy: batch data-parallel over 8 cores (2 batch rows per core). Per core:
  - host pre-transposes x to [b, d, t] so the contraction dim lands on SBUF
    partitions; weights pre-transposed likewise.
  - matmul1 on TensorE produces xp tiles [h=128, t=512] in PSUM. In fp8 mode
    (default) it runs as 3 DoubleRow fp8 passes (0.5 cyc/row, both d-blocks
    per pass): W_hi@x_hi + W_hi@x_lo + W_lo@x_hi, where x and W_in are
    residual-split into e4m3 hi+lo pairs ON THE HOST with scales (8, 64)
    chosen to keep the lo parts out of the fp8 subnormal range. The dropped
    W_lo@x_lo term is O(eps^2). The 1/512 scale folds into W_out.
  - VectorE tensor_tensor_scan runs the recurrence along the free (t) axis
    with the per-h decay broadcast from a [128,1] column, carry chained
    across t-chunks via the previous tile's last column (hs stays f32r:
    chunk-boundary rounding of the carry must stay well above bf16),
  - matmul2 on TensorE contracts h back to d_out in f32r, ScalarE copies
    PSUM->SBUF, out [b, o, t] DMAs back, host transposes to [b, t, o].
"""
from contextlib import ExitStack

import numpy as np

import concourse.bass as bass
import concourse.mybir as mybir
import concourse.tile as tile
from concourse import bacc
from concourse.bass_utils import run_bass_kernel_spmd

B, T, D_IN, HID, D_OUT = 16, 4096, 256, 512, 256
NCORES = 8
BPC = B // NCORES          # batch rows per core
TC = 512                   # t-chunk (PSUM bank = 512 fp32)
NCH = T // TC
ND = D_IN // 128           # 2  d-blocks
NH = HID // 128            # 4  h-blocks
NO = D_OUT // 128          # 2  o-blocks

SX = 8.0                   # host scale on x before fp8 split
SW = 64.0                  # host scale on W_in before fp8 split

# 'fp8'  : mm1 as 3 fp8-e4m3 DoubleRow passes (hi/lo residual split), mm2 f32r
# 'f32r' : fp32 storage, PE runs reduced-precision single-pass (1 cyc/row)
MODE_DEFAULT = "fp8"

# schedule/tuning knobs (read by _build; cache key includes them)
CFG = dict(sched="zip", xp_bufs=6, op_bufs=2, hs_bufs=16,
           x_first=512, head_split=1, tail_split=2, warmup=5)

_cache: dict = {}


def _build_fp8() -> bass.Bass:
    f32 = mybir.dt.float32
    f32r = mybir.dt.float32r
    f8 = mybir.dt.float8e4
    DR = mybir.MatmulPerfMode.DoubleRow

    nc = bacc.Bacc(None, target_bir_lowering=False)

    # x/weight layouts are pre-arranged on the host so every DMA below is a
    # single instruction with identical src/dst iteration order and >=512B
    # contiguous runs (HWDGE charges ~625ns fixed per DMA instruction, so
    # instruction count matters more than transfer size).
    x8 = nc.declare_dram_parameter("x8", [BPC, 128, 2, ND, T], f8, isOutput=False)
    w8 = nc.declare_dram_parameter("w8", [128, 2, ND, HID], f8, isOutput=False)
    w_outT = nc.declare_dram_parameter("w_outT", [128, NH, D_OUT], f32r,
                                       isOutput=False)
    dcols = nc.declare_dram_parameter("dcols", [128, NH], f32, isOutput=False)
    out = nc.declare_dram_parameter("out", [BPC, D_OUT, T], f32, isOutput=True)

    with tile.TileContext(nc) as tc, ExitStack() as ctx:
        const_pool = ctx.enter_context(tc.tile_pool(name="const", bufs=1))
        x_pool = ctx.enter_context(tc.tile_pool(name="xt", bufs=2 * BPC))
        o_pool = ctx.enter_context(tc.tile_pool(name="ot", bufs=6))
        hs_pool = ctx.enter_context(tc.tile_pool(name="hs", bufs=CFG["hs_bufs"]))
        xp_psum = ctx.enter_context(
            tc.tile_pool(name="xp", bufs=CFG["xp_bufs"], space=bass.MemorySpace.PSUM))
        op_psum = ctx.enter_context(
            tc.tile_pool(name="op", bufs=CFG["op_bufs"], space=bass.MemorySpace.PSUM))

        # scratch warmup matmuls: burn the PE pstate ramp (~3us at reduced
        # clock) during the head DMA latency so real matmuls run full speed.
        # Operands are never initialized or read back; only timing matters.
        nwarm = CFG.get("warmup", 0)
        if nwarm:
            warm = const_pool.tile([128, 512], mybir.dt.bfloat16, tag="warm")
            nc.gpsimd.memset(warm[:], 0.0)
            wpsum = xp_psum.tile([128, 512], f32, name="wu", tag="xp")
            for _ in range(nwarm):
                nc.tensor.matmul(wpsum[:], warm[:, 0:128], warm[:],
                                 start=True, stop=True)

        x8t = {}
        for b in range(BPC):
            x8t[b] = x_pool.tile([128, 2, ND, T], f8, name="x8t", tag="x8t")

        def load_x(b, csl, eng=None):
            (eng or nc.sync).dma_start(x8t[b][:, :, :, csl],
                                       x8[b, :, :, :, csl])

        # DMA order = dispatch order per queue; HWDGE serializes ~625ns per
        # DMA globally, so the head spreads the first loads across engine
        # queues (issue in parallel) and keeps them small. x loads interleave
        # batches to match the zip compute order.
        # Head loads: pass-1/2 operands first. x-lo on the Act queue is safe
        # here (no copies queued yet); all in-loop loads go on SP so the Act
        # queue never head-of-line-blocks stage2's PSUM-freeing copies.
        XF = CFG["x_first"]
        w8_sb = const_pool.tile([128, 2, ND, HID], f8, tag="w8")
        nc.sync.dma_start(w8_sb[:], w8[:])
        load_x(0, slice(0, XF))                           # SP
        dc = const_pool.tile([128, NH], f32, tag="dc")
        nc.scalar.dma_start(dc[:], dcols[:])
        load_x(1, slice(0, XF), nc.scalar)
        wo_sb = const_pool.tile([128, NH, D_OUT], f32r, tag="wo")
        nc.scalar.dma_start(wo_sb[:], w_outT[:])

        # per-batch chunk lists; the globally-last chunks are split small so
        # the final scan->matmul2 dependency chain drains quickly
        chunks = {}
        for b in range(BPC):
            sizes = [TC] * NCH
            hs_ = CFG.get("head_split", 1)
            if hs_ > 1:
                sizes = [TC // hs_] * hs_ + sizes[1:]
            if b == BPC - 1 and CFG["tail_split"] > 1:
                ts = CFG["tail_split"]
                sizes = sizes[:-1] + [TC // ts] * ts
            lst, t0 = [], 0
            for L in sizes:
                lst.append((t0, L))
                t0 += L
            chunks[b] = lst

        prev_hs = {}

        def stage1(b, ic):
            """matmul1 (3 fp8 DoubleRow passes) + scan per h-block."""
            t0, L = chunks[b][ic]
            tsl = slice(t0, t0 + L)
            for hblk in range(NH):
                hsl = slice(hblk * 128, (hblk + 1) * 128)
                # the very first chunk borrows the (still idle) op pool for
                # two of its xp tiles, deepening the early matmul1 pipeline
                pool = (op_psum if (b == 0 and ic == 0 and hblk < 2)
                        else xp_psum)
                xp = pool.tile([128, L], f32, name="xp",
                               tag="op" if pool is op_psum else "xp")
                nc.tensor.matmul(xp[:], w8_sb[:, 0, :, hsl],
                                 x8t[b][:, 0, :, tsl],
                                 start=True, stop=False, perf_mode=DR)
                nc.tensor.matmul(xp[:], w8_sb[:, 1, :, hsl],
                                 x8t[b][:, 0, :, tsl],
                                 start=False, stop=False, perf_mode=DR)
                nc.tensor.matmul(xp[:], w8_sb[:, 0, :, hsl],
                                 x8t[b][:, 1, :, tsl],
                                 start=False, stop=True, perf_mode=DR)
                hs = hs_pool.tile([128, L], f32r, name="hs", tag="hs")
                if ic == 0:
                    init = 0.0
                else:
                    pL = chunks[b][ic - 1][1]
                    init = prev_hs[(b, ic - 1, hblk)][:, pL - 1:pL]
                nc.vector.tensor_tensor_scan(
                    hs[:], dc[:, hblk:hblk + 1].to_broadcast((128, L)),
                    xp[:], init,
                    op0=mybir.AluOpType.mult, op1=mybir.AluOpType.add)
                prev_hs[(b, ic, hblk)] = hs

        def stage2(b, ic, tail=False):
            """matmul2 (f32r) + PSUM->SBUF copy (+ out DMA) per o-block.

            The last batch flushes its output per chunk (on the scalar
            queue, right after the staging copy) so the final DMA after the
            last matmul2 is small; earlier batches batch up OP columns per
            DMA and overlap with later compute.
            """
            t0, L = chunks[b][ic]
            for oblk in range(NO):
                op = op_psum.tile([128, L], f32, name="op", tag="op")
                for hblk in range(NH):
                    nc.tensor.matmul(
                        op[:],
                        wo_sb[:, hblk, oblk * 128:(oblk + 1) * 128],
                        prev_hs[(b, ic, hblk)][:],
                        start=(hblk == 0), stop=(hblk == NH - 1))
                st = o_pool.tile([128, L], f32, name="ot", tag="ot")
                nc.scalar.copy(st[:], op[:])
                # SWDGE (Pool engine) queue: keeps flush DMAs off the SP/Act
                # queues so data-waits here never block x loads or copies.
                # The last units alternate onto SP (idle by then) so the
                # final flushes don't serialize on one queue.
                if tail == 2:          # very last unit: nothing queued after
                    eng = nc.sync if oblk == 0 else nc.scalar
                elif tail:
                    eng = nc.sync if oblk == 0 else nc.gpsimd
                else:
                    eng = nc.gpsimd
                eng.dma_start(
                    out[b, oblk * 128:(oblk + 1) * 128, t0:t0 + L],
                    st[:])

        # interleave batches: the two scan chains are independent, so PE
        # always has the other batch's matmul1 to run while one batch's
        # scans complete; PE (not the scan chain) paces the pipeline.
        lists = [[(b, ic) for ic in range(len(chunks[b]))]
                 for b in range(BPC)]
        order = []
        k = 0
        while any(k < len(lst) for lst in lists):
            for lst in lists:
                if k < len(lst):
                    order.append(lst[k])
            k += 1

        # software-paced prefetch: x pieces are emitted on SP interleaved
        # with the flush DMAs, a few units ahead of their consumer
        loaded = {b: XF for b in range(BPC)}

        def prefetch(k):
            if k >= len(order):
                return
            b, ic = order[k]
            t0, L = chunks[b][ic]
            if t0 + L > loaded[b]:
                load_x(b, slice(loaded[b], t0 + L))
                loaded[b] = t0 + L

        PF = CFG.get("prefetch", 4)
        for j in range(PF):
            prefetch(j)
        # software pipeline: keep next chunk's matmul1s ahead of the
        # scan-dependent matmul2s in PE program order, across batches
        stage1(*order[0])
        ntail = len(order) - 4
        for k in range(len(order) - 1):
            prefetch(k + PF)
            stage1(*order[k + 1])
            stage2(*order[k], tail=(1 if k >= ntail else 0))
        stage2(*order[-1], tail=2)

    nc.compile()
    return nc


def _build_f32r() -> bass.Bass:
    """Baseline f32r variant (kept as fallback)."""
    f32 = mybir.dt.float32
    dt_in = mybir.dt.float32r
    dt_hs = dt_in

    nc = bacc.Bacc(None, target_bir_lowering=False)

    xT = nc.declare_dram_parameter("xT", [BPC, D_IN, T], dt_in, isOutput=False)
    w_inT = nc.declare_dram_parameter("w_inT", [D_IN, HID], dt_in, isOutput=False)
    w_outT = nc.declare_dram_parameter("w_outT", [HID, D_OUT], dt_in, isOutput=False)
    dcols = nc.declare_dram_parameter("dcols", [128, NH], f32, isOutput=False)
    out = nc.declare_dram_parameter("out", [BPC, D_OUT, T], f32, isOutput=True)

    with tile.TileContext(nc) as tc, ExitStack() as ctx:
        const_pool = ctx.enter_context(tc.tile_pool(name="const", bufs=1))
        x_pool = ctx.enter_context(tc.tile_pool(name="xt", bufs=BPC * ND))
        o_pool = ctx.enter_context(tc.tile_pool(name="ot", bufs=8))
        hs_pool = ctx.enter_context(tc.tile_pool(name="hs", bufs=CFG["hs_bufs"]))
        xp_psum = ctx.enter_context(
            tc.tile_pool(name="xp", bufs=CFG["xp_bufs"], space=bass.MemorySpace.PSUM))
        op_psum = ctx.enter_context(
            tc.tile_pool(name="op", bufs=CFG["op_bufs"], space=bass.MemorySpace.PSUM))

        XP_LEN = 512
        xt = {}
        for b in range(BPC):
            for dblk in range(ND):
                xt[(b, dblk)] = x_pool.tile([128, T], dt_in, name="xt", tag="xt")

        def load_x(b, dblk, piece):
            psl = slice(piece * XP_LEN, (piece + 1) * XP_LEN)
            nc.sync.dma_start(xt[(b, dblk)][:, psl],
                              xT[b, dblk * 128:(dblk + 1) * 128, psl])

        for dblk in range(ND):
            load_x(0, dblk, 0)
        wi = []
        for dblk in range(ND):
            w = const_pool.tile([128, HID], dt_in, tag=f"wi{dblk}")
            nc.sync.dma_start(w[:], w_inT[dblk * 128:(dblk + 1) * 128, :])
            wi.append(w)
        wo = []
        for hblk in range(NH):
            w = const_pool.tile([128, D_OUT], dt_in, tag=f"wo{hblk}")
            nc.sync.dma_start(w[:], w_outT[hblk * 128:(hblk + 1) * 128, :])
            wo.append(w)
        dc = const_pool.tile([128, NH], f32, tag="dc")
        nc.sync.dma_start(dc[:], dcols[:])
        for piece in range(1, T // XP_LEN):
            for dblk in range(ND):
                load_x(0, dblk, piece)
        for b in range(1, BPC):
            for piece in range(T // XP_LEN):
                for dblk in range(ND):
                    load_x(b, dblk, piece)

        OP = CFG["out_piece"]
        ot = {}
        prev_hs = {}

        def stage1(b, ic):
            tsl = slice(ic * TC, (ic + 1) * TC)
            for hblk in range(NH):
                xp = xp_psum.tile([128, TC], f32, name="xp", tag="xp")
                for dblk in range(ND):
                    nc.tensor.matmul(
                        xp[:],
                        wi[dblk][:, hblk * 128:(hblk + 1) * 128],
                        xt[(b, dblk)][:, tsl],
                        start=(dblk == 0), stop=(dblk == ND - 1))
                hs = hs_pool.tile([128, TC], dt_hs, name="hs", tag="hs")
                init = (0.0 if ic == 0
                        else prev_hs[(b, ic - 1, hblk)][:, TC - 1:TC])
                nc.vector.tensor_tensor_scan(
                    hs[:], dc[:, hblk:hblk + 1].to_broadcast((128, TC)),
                    xp[:], init,
                    op0=mybir.AluOpType.mult, op1=mybir.AluOpType.add)
                prev_hs[(b, ic, hblk)] = hs

        def stage2(b, ic, tail=False):
            q, csl = divmod(ic * TC, OP)
            for oblk in range(NO):
                op = op_psum.tile([128, TC], f32, name="op", tag="op")
                for hblk in range(NH):
                    nc.tensor.matmul(
                        op[:],
                        wo[hblk][:, oblk * 128:(oblk + 1) * 128],
                        prev_hs[(b, ic, hblk)][:],
                        start=(hblk == 0), stop=(hblk == NH - 1))
                if csl == 0:
                    ot[(b, oblk)] = o_pool.tile([128, OP], f32,
                                                name="ot", tag="ot")
                nc.scalar.copy(ot[(b, oblk)][:, csl:csl + TC], op[:])
                if csl + TC == OP:
                    nc.sync.dma_start(
                        out[b, oblk * 128:(oblk + 1) * 128,
                            q * OP:(q + 1) * OP],
                        ot[(b, oblk)][:])

        for b in range(BPC):
            stage1(b, 0)
            for ic in range(NCH - 1):
                stage1(b, ic + 1)
                stage2(b, ic)
            stage2(b, NCH - 1)

    nc.compile()
    return nc


def _build(mode: str) -> bass.Bass:
    if mode == "fp8":
        return _build_fp8()
    return _build_f32r()


def _prep_inputs(x, W_in, W_h, W_out, mode: str):
    import ml_dtypes
    f8 = ml_dtypes.float8_e4m3
    d = np.ascontiguousarray(np.diagonal(np.asarray(W_h, np.float32)))
    dcols = np.ascontiguousarray(d.reshape(NH, 128).T, dtype=np.float32)

    if mode == "fp8":
        xs = np.transpose(np.asarray(x, np.float32), (0, 2, 1)) * SX  # [B,D,T]
        xh = xs.astype(f8)
        xl = (xs - xh.astype(np.float32)).astype(f8)
        # [B, D, T] -> [B, 128, 2(hl), ND, T] to match the SBUF tile layout
        xh = xh.reshape(B, ND, 128, T).transpose(0, 2, 1, 3)
        xl = xl.reshape(B, ND, 128, T).transpose(0, 2, 1, 3)
        x8 = np.ascontiguousarray(np.stack([xh, xl], axis=2))
        ws = np.asarray(W_in, np.float32).T * SW                      # [D,H]
        wh = ws.astype(f8)
        wl = (ws - wh.astype(np.float32)).astype(f8)
        wh = wh.reshape(ND, 128, HID).transpose(1, 0, 2)
        wl = wl.reshape(ND, 128, HID).transpose(1, 0, 2)
        w8 = np.ascontiguousarray(np.stack([wh, wl], axis=1))
        w_outT = np.ascontiguousarray(
            (np.asarray(W_out, np.float32).T / (SX * SW))
            .reshape(NH, 128, D_OUT).transpose(1, 0, 2))
        in_maps = []
        for c in range(NCORES):
            in_maps.append({
                "x8": np.ascontiguousarray(x8[c * BPC:(c + 1) * BPC]),
                "w8": w8,
                "w_outT": w_outT,
                "dcols": dcols,
            })
        return in_maps

    xT = np.ascontiguousarray(
        np.transpose(np.asarray(x, np.float32), (0, 2, 1)))
    w_inT = np.ascontiguousarray(np.asarray(W_in, np.float32).T)
    w_outT = np.ascontiguousarray(np.asarray(W_out, np.float32).T)
    in_maps = []
    for c in range(NCORES):
        in_maps.append({
            "xT": np.ascontiguousarray(xT[c * BPC:(c + 1) * BPC]),
            "w_inT": w_inT,
            "w_outT": w_outT,
            "dcols": dcols,
        })
    return in_maps


def _get_nc(mode: str = MODE_DEFAULT):
    key = (mode, tuple(sorted(CFG.items())))
    if key not in _cache:
        _cache[key] = _build(mode)
    return _cache[key]


def _run(x, W_in, W_h, W_out, mode: str = MODE_DEFAULT, **spmd_kwargs):
    nc = _get_nc(mode)
    in_maps = _prep_inputs(x, W_in, W_h, W_out, mode)
    res = run_bass_kernel_spmd(nc, in_maps, list(range(NCORES)), **spmd_kwargs)
    parts = [np.transpose(np.asarray(res.results[c]["out"]), (0, 2, 1))
             for c in range(NCORES)]
    full = np.concatenate(parts, axis=0).astype(np.float32)
    return full, res


def kernel(x, W_in, W_h, W_out):
    out, _ = _run(x, W_in, W_h, W_out)
    return out
